# revision 1
# baseline (speedup 1.0000x reference)
"""Multi-head attention Trainium2 kernel, head-sharded across 8 NeuronCores.

Problem (hardcoded): B=4, S=2048, E=1024, H=8, D=128, fp32.
  q = xh @ Wq[h].T + bq[h]; k,v likewise
  out = softmax((q @ k.T) / sqrt(H)) @ v, concat heads.

Sharding: head h -> core h. Each core gets x_h^T [D, B*S] (host-transposed,
fp32 + bf16 copies), per-head weights (transposed to matmul layout), and
writes out_h [B*S, D]. No collectives; host concatenates per-core outputs
along the feature dim.

Per-core algorithm (scores kept transposed so no PE transposes are needed):
  q_dT[o, t] = Wq.T-matmul over xT        (fp32r, full-rate, [128, 8192] SBUF)
  k_dT[o, t] likewise
  v[t, o] per 128-token tile (bf16 matmul), augmented with a ones column
    -> v_aug [128, 129] bf16
  per batch b, per 1024-wide query block:
    for each key tile t (16 of 128):
      scores_T psum[t-sub, sq] = k-tile^T-matmul vs q block   (fp32r)
      attn_T = exp(scores_T / sqrt(8))  via ACT psum->sbuf, bf16
      for each 128-query subtile j: psum_out[j] += attn_T_j^T-matmul v_aug[t]
        (col 128 of psum_out accumulates the softmax denominator)
    normalize: out[j] = psum_out[j][:, :128] * (1 / psum_out[j][:, 128])
No row-max subtraction: scores/sqrt(8) stay < ~30, exp fits fp32/bf16 range.
Projections are interleaved per batch so PE has fill work during the
per-block psum drain, and attention can start before all batches project.
"""

import contextlib
import math

import numpy as np

import concourse.bacc as bacc
import concourse.bass as bass
import concourse.mybir as mybir
import concourse.tile as tile

B, S, E, H, D = 4, 2048, 1024, 8, 128
T = B * S  # tokens per core (all batches)
SCALE = 1.0 / math.sqrt(H)

F32 = mybir.dt.float32
F32R = mybir.dt.float32r  # fp32 storage, full-rate matmul for moving dim >= 256
BF16 = mybir.dt.bfloat16

N_CORES = 8
SQ_BLK = 1024  # query block width
N_TT = S // 128  # key tiles per batch (16)
N_SQB = S // SQ_BLK  # query blocks per batch (2)
N_J = SQ_BLK // 128  # query subtiles per block (8)
VA = 129  # v tile free width (128 features + ones column)
GRP = (3, 3, 2)  # psum_out packing: 8 accumulators in 3 one-bank tiles

# Note: offloading part of the exp to the VectorEngine (Schraudolph bits =
# scores*a+b written int16, bitcast bf16) was tried and measured on HW: it
# rebalances engine busy (ACT 138->105us) but does NOT improve e2e (200 vs
# 198us) because the bottleneck is the per-iteration scores->exp->attnV
# dependency chain plus PE weight-load overhead, not ACT saturation -- and
# it costs accuracy (rel err 2.6e-3 -> 4.3e-3). Kept out.

_CACHE = {}


def _build_body(ctx, tc, loop_k=1):
    nc = tc.nc
    xT = nc.dram_tensor("xT", [D, T], F32R, kind="ExternalInput").ap()
    xTb = nc.dram_tensor("xTb", [D, T], BF16, kind="ExternalInput").ap()
    wqT = nc.dram_tensor("wqT", [D, D], F32R, kind="ExternalInput").ap()
    wkT = nc.dram_tensor("wkT", [D, D], F32R, kind="ExternalInput").ap()
    wvT = nc.dram_tensor("wvT", [D, D], BF16, kind="ExternalInput").ap()
    bqv = nc.dram_tensor("bq", [D, 1], F32, kind="ExternalInput").ap()
    bkv = nc.dram_tensor("bk", [D, 1], F32, kind="ExternalInput").ap()
    bvb = nc.dram_tensor("bvb", [D, D], F32, kind="ExternalInput").ap()
    out = nc.dram_tensor("out", [T, D], F32, kind="ExternalOutput").ap()

    singles = ctx.enter_context(tc.tile_pool(name="singles", bufs=1))
    ps_pool = ctx.enter_context(tc.tile_pool(name="ps", bufs=2, space="PSUM"))
    po_pool = ctx.enter_context(tc.tile_pool(name="po", bufs=1, space="PSUM"))
    at_pool = ctx.enter_context(tc.tile_pool(name="at", bufs=16))
    o_pool = ctx.enter_context(tc.tile_pool(name="osb", bufs=8))
    r_pool = ctx.enter_context(tc.tile_pool(name="rec", bufs=8))

    # persistent SBUF
    xT_sb = singles.tile([D, T], F32R, tag="xT")
    xTb_sb = singles.tile([D, T], BF16, tag="xTb")
    wq_sb = singles.tile([D, D], F32R, tag="wq")
    wk_sb = singles.tile([D, D], F32R, tag="wk")
    wv_sb = singles.tile([D, D], BF16, tag="wv")
    bq_sb = singles.tile([D, 1], F32, tag="bq")
    bk_sb = singles.tile([D, 1], F32, tag="bk")
    bvb_sb = singles.tile([D, D], F32, tag="bvb")
    q_sb = singles.tile([D, T], F32R, tag="q")
    k_sb = singles.tile([D, T], F32R, tag="k")
    vaug = [
        singles.tile([128, N_TT * VA], BF16, tag=f"va{b}", name=f"vaug{b}")
        for b in range(B)
    ]

    # dummy exp at program start: pulls the one-time ~2.7us ACT table load
    # under the initial input DMA (ACT is otherwise idle there), so a cold
    # single-shot run doesn't pay it on the first real activation
    warm = singles.tile([128, 1], F32, tag="warm")
    nc.vector.memset(warm[:], 0.0)
    nc.scalar.activation(warm[:], warm[:], mybir.ActivationFunctionType.Exp)

    nc.sync.dma_start(wq_sb[:], wqT)
    nc.sync.dma_start(wk_sb[:], wkT)
    nc.sync.dma_start(wv_sb[:], wvT)
    nc.sync.dma_start(bq_sb[:], bqv)
    nc.sync.dma_start(bk_sb[:], bkv)
    nc.sync.dma_start(bvb_sb[:], bvb)

    def emit_qk_chunks(b, chunks, which="qk"):
        b0 = b * S
        fast = b == 0  # batch 0 gates the whole pipeline start
        pairs = []
        if "k" in which:
            pairs.append((wk_sb, bk_sb, k_sb))
        if "q" in which:
            pairs.append((wq_sb, bq_sb, q_sb))
        for w_sb, b_sb, dst in pairs:
            for c in chunks:
                sl = slice(b0 + c * 512, b0 + (c + 1) * 512)
                if fast:
                    # "s" slots are free before attention starts; ACT Identity
                    # (same table set as Exp) + DVE split the psum->sbuf copies
                    ps = ps_pool.tile([128, SQ_BLK], F32, tag="s", name="ps")
                else:
                    ps = ps_pool.tile([128, 512], F32, tag="pp", bufs=1, name="pp")
                if fast and c == 0:
                    # first chunk in two N=256 matmuls (fp32r full-rate at
                    # >=256) so compute starts after the half-size DMA
                    nc.tensor.matmul(
                        ps[:, 0:256],
                        w_sb[:],
                        xT_sb[:, sl.start : sl.start + 256],
                        start=True,
                        stop=True,
                    )
                    nc.tensor.matmul(
                        ps[:, 256:512],
                        w_sb[:],
                        xT_sb[:, sl.start + 256 : sl.stop],
                        start=True,
                        stop=True,
                        skip_group_check=True,
                    )
                else:
                    nc.tensor.matmul(
                        ps[:, 0:512], w_sb[:], xT_sb[:, sl], start=True, stop=True
                    )
                if fast and c % 2 == 0:
                    nc.scalar.activation(
                        dst[:, sl],
                        ps[:, 0:512],
                        mybir.ActivationFunctionType.Identity,
                        bias=b_sb[:],
                    )
                else:
                    nc.vector.tensor_scalar_add(dst[:, sl], ps[:, 0:512], b_sb[:])

    def emit_v(b):
        b0 = b * S
        groups = range(N_TT // 4)
        # 4 V-tiles share one psum bank (each matmul is a start+stop single
        # writing a disjoint 128-col region; the bank-wide has_written clear
        # doesn't disturb sibling DATA) so the 4 matmuls run back-to-back
        # instead of ping-ponging with the DVE copy.
        for g in groups:
            ps = ps_pool.tile([128, 512], F32, tag="pp", bufs=1, name="pp")
            for m in range(4):
                i = g * 4 + m
                t0 = b0 + i * 128
                nc.tensor.matmul(
                    ps[:, m * 128 : (m + 1) * 128],
                    xTb_sb[:, t0 : t0 + 128],
                    wv_sb[:],
                    start=True,
                    stop=True,
                    skip_group_check=True,
                )
            for m in range(4):
                i = g * 4 + m
                nc.vector.tensor_add(
                    vaug[b][:, i * VA : i * VA + 128],
                    ps[:, m * 128 : (m + 1) * 128],
                    bvb_sb[:],
                )

    def emit_loads(b):
        b0 = b * S
        if b == 0:
            # split the very first chunk so the K projection (and therefore
            # the whole attention pipeline) starts after a half-size DMA
            nc.sync.dma_start(xT_sb[:, b0 : b0 + 256], xT[:, b0 : b0 + 256])
            nc.sync.dma_start(xT_sb[:, b0 + 256 : b0 + 512], xT[:, b0 + 256 : b0 + 512])
        else:
            nc.sync.dma_start(xT_sb[:, b0 : b0 + 512], xT[:, b0 : b0 + 512])
        for c in range(1, 4):
            sl = slice(b0 + c * 512, b0 + (c + 1) * 512)
            nc.sync.dma_start(xT_sb[:, sl], xT[:, sl])
        nc.sync.dma_start(xTb_sb[:, b0 : b0 + S], xTb[:, b0 : b0 + S])
        va_v = vaug[b][:].rearrange("p (n c) -> p n c", c=VA)
        nc.vector.memset(va_v[:, :, 128:129], 1.0)

    def emit_proj(b):
        emit_loads(b)
        emit_qk_chunks(b, range(4), which="kq")
        emit_v(b)

    def emit_attn_block(b, sqb):
        sq0 = b * S + sqb * SQ_BLK
        grp = [
            po_pool.tile([128, VA * n], F32, tag=f"po{g}", name=f"po{g}")
            for g, n in enumerate(GRP)
        ]

        def po_slice(j):
            g, m = (j // 3, j % 3) if j < 6 else (2, j - 6)
            return grp[g][:, m * VA : (m + 1) * VA]

        def emit_attnv(t, at):
            for j in range(N_J):
                # start=True clears has_written for the WHOLE bank, so only
                # the first slice packed into each bank may set it; sibling
                # slices overwrite-where-unset on t==0 and accumulate after.
                first_in_bank = j in (0, 3, 6)
                nc.tensor.matmul(
                    po_slice(j),
                    at[:, j * 128 : (j + 1) * 128],
                    vaug[b][:, t * VA : (t + 1) * VA],
                    start=(t == 0 and first_in_bank),
                    stop=(t == N_TT - 1),
                    skip_group_check=True,
                )

        # attnV(t-1) is emitted after scores(t)/exp(t) so the PE prefers
        # feeding ACT's next input over draining the previous tile.
        # (512-wide half-block exps were tried: model prices ACT overhead
        # higher and HW measured a noisier 196-213us band vs 198us here.)
        pending = None  # (t, at) awaiting attnV
        for t in range(N_TT):
            ksl = k_sb[:, b * S + t * 128 : b * S + (t + 1) * 128]
            ps = ps_pool.tile([128, SQ_BLK], F32, tag="s", name="ps")
            nc.tensor.matmul(
                ps[:, 0:512],
                ksl,
                q_sb[:, sq0 : sq0 + 512],
                start=True,
                stop=True,
            )
            nc.tensor.matmul(
                ps[:, 512:1024],
                ksl,
                q_sb[:, sq0 + 512 : sq0 + 1024],
                start=True,
                stop=True,
            )
            at = at_pool.tile([128, SQ_BLK], BF16, tag="at", name="at")
            nc.scalar.activation(
                at[:], ps[:], mybir.ActivationFunctionType.Exp, scale=SCALE
            )
            if pending is not None:
                emit_attnv(*pending)
            pending = (t, at)
        emit_attnv(*pending)

        # drain + normalize + store; the very last block normalizes on ACT
        # (idle at the tail) to shorten the epilogue
        last = b == B - 1 and sqb == N_SQB - 1
        for g, n in enumerate(GRP):
            gv = grp[g][:].rearrange("p (n c) -> p n c", c=VA)
            rec = r_pool.tile([128, 4], F32, tag="rec", name="rec")
            nc.vector.reciprocal(
                rec[:, 0:n].rearrange("p (n one) -> p n one", one=1),
                gv[:, :, 128:129],
            )
            for m in range(n):
                j = g * 3 + m
                o_sb = o_pool.tile([128, 128], F32, tag="o", name="o_sb")
                if last:
                    nc.scalar.activation(
                        o_sb[:],
                        grp[g][:, m * VA : m * VA + 128],
                        mybir.ActivationFunctionType.Identity,
                        scale=rec[:, m : m + 1],
                    )
                else:
                    nc.vector.tensor_scalar_mul(
                        o_sb[:], grp[g][:, m * VA : m * VA + 128], rec[:, m : m + 1]
                    )
                r0 = sq0 + j * 128
                nc.sync.dma_start(out[r0 : r0 + 128, :], o_sb[:])

    def emit_body():
        # proj(b+1) is emitted between batch b's two attention blocks: the
        # scheduler fills PE slack (ACT-bound attention + drain boundary)
        # with next batch's projections instead of stalling ACT between
        # batches. Batch 0's Q chunks 2-3 (only needed by its second block)
        # are deferred past the first block so the first scores matmul isn't
        # stuck behind their psum slot cycle.
        emit_proj(0)
        for b in range(B):
            emit_attn_block(b, 0)
            if b + 1 < B:
                emit_proj(b + 1)
            emit_attn_block(b, 1)

    if loop_k > 1:
        # hint_engines: the body far exceeds one IRAM block per engine, so
        # prefetch the back-edge target to avoid a ~4us ifetch stall per
        # iteration (measurement loop only).
        hints = (
            mybir.EngineType.PE,
            mybir.EngineType.Activation,
            mybir.EngineType.DVE,
            mybir.EngineType.SP,
        )
        with tc.For_i(0, loop_k, 1, hint_engines=hints):
            emit_body()
    else:
        emit_body()


def build(loop_k=1):
    nc = bacc.Bacc(
        "TRN2",
        target_bir_lowering=False,
        debug=False,
        enable_asserts=False,
        num_devices=N_CORES,
    )
    with tile.TileContext(nc) as tc:
        with contextlib.ExitStack() as ctx:
            _build_body(ctx, tc, loop_k=loop_k)
    nc.compile()
    return nc


def get_nc():
    if "nc" not in _CACHE:
        _CACHE["nc"] = build()
    return _CACHE["nc"]


def make_in_maps(sequences, Wq, Wk, Wv, bq, bk, bv):
    import ml_dtypes

    sequences = np.asarray(sequences, dtype=np.float32)
    Wq = np.asarray(Wq, dtype=np.float32)
    Wk = np.asarray(Wk, dtype=np.float32)
    Wv = np.asarray(Wv, dtype=np.float32)
    bq = np.asarray(bq, dtype=np.float32)
    bk = np.asarray(bk, dtype=np.float32)
    bv = np.asarray(bv, dtype=np.float32)
    in_maps = []
    for h in range(N_CORES):
        xh = sequences[:, :, h * D : (h + 1) * D].reshape(T, D)
        xT = np.ascontiguousarray(xh.T)
        in_maps.append(
            {
                "xT": xT,
                "xTb": xT.astype(ml_dtypes.bfloat16),
                "wqT": np.ascontiguousarray(Wq[h].T),
                "wkT": np.ascontiguousarray(Wk[h].T),
                "wvT": np.ascontiguousarray(Wv[h].T).astype(ml_dtypes.bfloat16),
                "bq": np.ascontiguousarray(bq[h].reshape(D, 1)),
                "bk": np.ascontiguousarray(bk[h].reshape(D, 1)),
                "bvb": np.ascontiguousarray(np.tile(bv[h][None, :], (D, 1))),
            }
        )
    return in_maps


def assemble(results):
    out = np.empty((B, S, E), np.float32)
    for h in range(N_CORES):
        out[:, :, h * D : (h + 1) * D] = results[h]["out"].reshape(B, S, D)
    return out


def kernel(sequences, Wq, Wk, Wv, bq, bk, bv):
    from concourse.bass_utils import run_bass_kernel_spmd

    nc = get_nc()
    in_maps = make_in_maps(sequences, Wq, Wk, Wv, bq, bk, bv)
    r = run_bass_kernel_spmd(nc, in_maps, core_ids=list(range(N_CORES)))
    return assemble(r.results)

